# revision 35
# baseline (speedup 1.0000x reference)
"""Multi-head attention (B=16, N=1024, EM=768, H=12, d=64) on 8 TRN2 NeuronCores.

Strategy: data-parallel over batch (2 batches per core, zero collectives).
v2 schedule (vs v1 baseline at 533us): the whole kernel is organized so the
PE never idles (keeps the tensor engine in its high p-state) and the ACT
engine's exp chain is never gated by PE queue position:

  1. Scores-ahead software pipelining: the score matmuls for beat j+1 are
     emitted BEFORE exp(j), so when exp(j) ends, exp(j+1)'s input is long
     since ready -- exp never waits on beat-j fill/PV sitting ahead of it
     in the in-order PE queue (that wait was ~430ns/beat = ~40% of the
     baseline's attention time).
  2. Minimal pre-attention phase: only QK(b0) head-0-3 feature tiles +
     V(b0) run dense; attention(b0) starts ~20us in. All remaining
     projection work (QK(b0) heads 4-11, QK(b1), V(b1), then OP(b0)+
     norm(b0) during attention(b1)) is deadline-ordered fill inside the
     exp-gated beats.
  3. Per-beat fill debt is budgeted in ns-equivalent units including
     LDWEIGHTS cost so beats stay just above the exp duration -- PE
     stays continuously busy (full 2.4GHz p-state) without overpacking.

Numerics identical to baseline (fp8 DoubleRow scores from host-permuted
W_qk, fp16 everything else, exp without max-subtraction, rowsums via a
constant-1 V column, b_v/b_proj folded on host). fp8 anywhere else
(V proj, PV, OP) measurably blows the 2e-2 budget: attention output is a
near-uniform average of random-sign values, so per-element quantization
noise passes through at full magnitude instead of averaging out.
"""

import sys

if "/opt/trn_rl_repo" not in sys.path:
    sys.path.insert(0, "/opt/trn_rl_repo")

import numpy as np

from concourse import bacc, mybir, tile
from concourse.bass_utils import run_bass_kernel_spmd

F8 = mybir.dt.float8e4
F16 = mybir.dt.float16
F32 = mybir.dt.float32

B, N, EM = 16, 1024, 768
H, D = 12, 64
NCORES = 8
BL = B // NCORES          # batches per core
T = BL * N                # tokens per core
NT = T // 128             # 16 token tiles
NE = EM // 128            # 6 em tiles
NQC = 512                 # q-chunk width
NK = N // 128             # 8 k-tiles per batch
SCALE = 1.0 / np.sqrt(np.float32(D))

# PE work estimates in "effective columns" (~ns * 2.4), calibrated from the
# v1 hardware trace (per-matmul ~46ns fixed tax + stationary-switch bubbles).
EFF_MM512 = 625           # 512-col fp16 matmul (~260ns)
EFF_MM256 = 350
EFF_SPAIR = 1650          # score pair, both halves (~690ns)
EFF_SPAIR_DENSE = 2250    # dense fp8 score pair (2 accumulating mm per half)
EFF_PVPAIR = 1245         # fp16 PV pair (~520ns)
EFF_BC = 1000             # norm broadcast matmul (~420ns)
BEAT_EFF = 4400           # per-beat PE budget (~1830ns)


def build_nc():
    nc = bacc.Bacc("TRN2", target_bir_lowering=False, debug=False,
                   num_devices=NCORES)
    xt_d = nc.dram_tensor("xt", [EM, T], F16, kind="ExternalInput").ap()
    wqk_d = nc.dram_tensor("wqk", [EM, 2 * EM], F16, kind="ExternalInput").ap()
    bqkc_d = nc.dram_tensor("bqkc", [128, 2 * NE], F32,
                            kind="ExternalInput").ap()
    wv_d = nc.dram_tensor("wv", [EM, EM], F16, kind="ExternalInput").ap()
    wp_d = nc.dram_tensor("wp", [EM, EM], F16, kind="ExternalInput").ap()
    sel_d = nc.dram_tensor("sel", [H, NE * 128], F16, kind="ExternalInput").ap()
    out_d = nc.dram_tensor("out", [T, EM], F16, kind="ExternalOutput").ap()

    with tile.TileContext(nc) as tc:
        with (
            tc.tile_pool(name="big", bufs=1) as big,
            tc.tile_pool(name="ptp", bufs=5) as ptp,
            tc.tile_pool(name="rap", bufs=1) as rap,
            tc.tile_pool(name="stgp", bufs=2) as stgp,
            tc.tile_pool(name="rcp", bufs=1) as rcp,
            tc.tile_pool(name="osb", bufs=2) as osbp,
            tc.tile_pool(name="bcsp", bufs=1) as bcsp,
            tc.tile_pool(name="ps_a", bufs=2, space="PSUM") as ps_a,
            tc.tile_pool(name="ps_pv", bufs=2, space="PSUM") as ps_pv,
            tc.tile_pool(name="ps_f", bufs=1, space="PSUM") as ps_f,
        ):
            # ---- constants ----
            sel = big.tile([H, NE, 128], F16)
            nc.sync.dma_start(out=sel, in_=sel_d)
            zb = big.tile([128, 1], F32)
            nc.vector.memset(zb, 0.0)

            wqk_sb = big.tile([128, NE, 2 * EM], F16)
            wv_sb = big.tile([128, NE, EM], F16)
            wp_sb = big.tile([128, NE, EM], F16)
            bqkc = big.tile([128, 2 * NE], F32)
            xT = big.tile([128, NE, T], F16)
            nc.sync.dma_start(out=bqkc, in_=bqkc_d)
            # DMA priority: P0's needs first (QK(b0) fts 0,1,6,7 + V(b0)),
            # then the rest in fill-deadline order. Issue from three
            # sequencers (sync/scalar/vector) so the queues stream in
            # parallel and the first QK matmul can start sooner.
            for e in range(NE):
                sl = slice(e * 128, (e + 1) * 128)
                nc.sync.dma_start(out=wqk_sb[:, e, 0:256],
                                  in_=wqk_d[sl, 0:256])
                nc.sync.dma_start(out=wqk_sb[:, e, 768:1024],
                                  in_=wqk_d[sl, 768:1024])
                if e == 0:
                    # halve the first tile's transfer so QK ft0's first
                    # matmul isn't gated on the full 256KB tile
                    nc.scalar.dma_start(out=xT[:, e, 0:NQC],
                                        in_=xt_d[sl, 0:NQC])
                    nc.scalar.dma_start(out=xT[:, e, NQC:N],
                                        in_=xt_d[sl, NQC:N])
                else:
                    nc.scalar.dma_start(out=xT[:, e, 0:N],
                                        in_=xt_d[sl, 0:N])
            for e in range(NE):
                sl = slice(e * 128, (e + 1) * 128)
                nc.sync.dma_start(out=wv_sb[:, e, :], in_=wv_d[sl, :])
                nc.scalar.dma_start(out=xT[:, e, N:T], in_=xt_d[sl, N:T])
            for e in range(NE):
                sl = slice(e * 128, (e + 1) * 128)
                nc.sync.dma_start(out=wqk_sb[:, e, 256:768],
                                  in_=wqk_d[sl, 256:768])
                nc.sync.dma_start(out=wqk_sb[:, e, 1024:1536],
                                  in_=wqk_d[sl, 1024:1536])
                nc.sync.dma_start(out=wp_sb[:, e, :], in_=wp_d[sl, :])

            # [32-row group, qk-pair, dgroup, tok]; head h lives at rows
            # 32*(h%4) of pair h//4 (Q) / 3+h//4 (K), d split as 2x32.
            # Matmul operand base partitions must be 0/32/64, so the
            # rows-96 head is DMA-duplicated into qk3 (base 0).
            qkT = big.tile([128, 2 * NE, 2, T], F8)
            qk3 = big.tile([128, 2, 2, T], F8)
            v4 = big.tile([128, NT, H, D + 1], F16)
            nc.vector.memset(v4[:, :, :, D:D + 1], 1.0)
            aoT = big.tile([128, NE, T], F16)

            def gen_qk(b, ft, pool, tag, scalar_copy=False):
                """QK projection for one feature tile; yields eff-cols."""
                bn = b * N
                ps = pool.tile([128, 2, NQC], F32, tag=tag, name="fqk")
                # e-outer: both halves share each e's stationary (one LDW)
                for e in range(NE):
                    for half in range(2):
                        csl = slice(bn + half * NQC, bn + (half + 1) * NQC)
                        nc.tensor.matmul(
                            ps[:, half, :],
                            wqk_sb[:, e, ft * 128:(ft + 1) * 128],
                            xT[:, e, csl],
                            start=(e == 0), stop=(e == NE - 1))
                        yield EFF_MM512
                for half in range(2):
                    csl = slice(bn + half * NQC, bn + (half + 1) * NQC)
                    dst = qkT[:, ft // 2, ft % 2, csl]
                    if scalar_copy and half == 1:
                        nc.scalar.activation(
                            dst, ps[:, half, :],
                            mybir.ActivationFunctionType.Identity,
                            bias=bqkc[:, ft:ft + 1])
                    else:
                        nc.vector.tensor_scalar_add(
                            dst, ps[:, half, :], bqkc[:, ft:ft + 1])
                if ft % 2 == 1:
                    fp = ft // 2
                    g0 = 32 * (fp % 3)
                    bsl = slice(bn, bn + N)
                    nc.sync.dma_start(
                        out=qk3[g0:g0 + 32, fp // 3, :, bsl],
                        in_=qkT[96:128, fp, :, bsl])

            def gen_v(b, kt, pool, tag, scalar_copy=False):
                tt = b * NK + kt
                tsl = slice(tt * 128, (tt + 1) * 128)
                ps = pool.tile([128, H, D], F32, tag=tag, name="fv")
                for e in range(NE):
                    for h0, h1 in ((0, 8), (8, 12)):
                        fsl = slice(h0 * D, h1 * D)
                        nc.tensor.matmul(
                            ps[:, h0:h1, :], xT[:, e, tsl],
                            wv_sb[:, e, fsl],
                            start=(e == 0), stop=(e == NE - 1))
                        yield EFF_MM512 if h1 - h0 == 8 else EFF_MM256
                if scalar_copy:
                    nc.scalar.copy(v4[:, tt, :, 0:D], ps)
                else:
                    nc.vector.tensor_copy(v4[:, tt, :, 0:D], ps)

            def gen_op(b, kt, pool, tag, scalar_copy=False):
                tt = b * NK + kt
                tsl = slice(tt * 128, (tt + 1) * 128)
                ps = pool.tile([128, EM], F32, tag=tag, name="fop")
                for dv in range(NE):
                    for c0, c1 in ((0, 512), (512, 768)):
                        nc.tensor.matmul(
                            ps[:, c0:c1], aoT[:, dv, tsl],
                            wp_sb[:, dv, c0:c1],
                            start=(dv == 0), stop=(dv == NE - 1))
                        yield EFF_MM512 if c1 - c0 == 512 else EFF_MM256
                osb = osbp.tile([128, EM], F16)
                if scalar_copy:
                    nc.scalar.copy(osb, ps)
                else:
                    nc.vector.tensor_copy(osb, ps)
                nc.sync.dma_start(out=out_d[tsl, :], in_=osb)

            def make_bc(rc):
                """All 12 recip-broadcast tiles via PE sel-matmuls, drained
                to fp16 SBUF in the idle phase-boundary window. The fill-time
                norm granules are then a pure DVE multiply (no PSUM held
                across the fill pipeline)."""
                bcsb = bcsp.tile([128, 2, NE, NQC], F16, name="bcsb")
                for i, (half, t) in enumerate(
                        [(hh, tt) for hh in range(2) for tt in range(NE)]):
                    pool, tag = ((ps_a, "mm") if i % 2 == 0
                                 else (ps_f, "fill"))
                    bc = pool.tile([128, NQC], F32, tag=tag, name="fbc")
                    nc.tensor.matmul(bc, sel[:, t, :], rc[:, half, :],
                                     start=True, stop=True)
                    if i % 2 == 1:
                        nc.scalar.copy(bcsb[:, half, t, :], bc)
                    else:
                        nc.vector.tensor_copy(bcsb[:, half, t, :], bc)
                return bcsb

            def gen_bc(b, half, t, bcsb):
                bn = b * N
                qsl = slice(bn + half * NQC, bn + (half + 1) * NQC)
                yield 50
                dst = aoT[:, t, qsl]
                nc.vector.tensor_mul(dst, dst, bcsb[:, half, t, :])

            fill_done = set()

            def chain(gens):
                for g in gens:
                    yield from g

            def tracked(gen, key):
                """Marks `key` done once the generator (incl. its trailing
                drain emissions) has fully run."""
                yield from gen
                fill_done.add(key)

            def run_all(g):
                for _ in g:
                    pass

            def emit_attention(b, rsall, fill, deadlines=()):
                """Software-pipelined beats: scores for beat j+1 are emitted
                before exp(j) so the ACT exp chain is never queued behind
                beat-j fill/PV work on the in-order PE."""
                bn = b * N
                beats = [(h, kt) for h in range(H) for kt in range(NK)]
                nb = len(beats)
                sps_of = {}
                pt_of = {}
                pvps_of = {}
                dense = [False]   # flips when fill runs dry: keep PE padded
                # PV(h,kt) is emitted at loop 8h+PV_AT[kt]: lag >=2 from its
                # exp (decouples ACT->PE), and head h's first pvp write lands
                # 3 beats after head h-1's drain was queued -- the 2-buf pvp
                # pool never stalls the PE at head boundaries.
                PV_AT = {0: 4, 1: 4, 2: 5, 3: 5, 4: 6, 5: 7, 6: 8, 7: 9}
                pv_sched = {}
                for hh in range(H):
                    for kk in range(NK):
                        pv_sched.setdefault(
                            8 * hh + PV_AT[kk], []).append(8 * hh + kk)

                def emit_scores(j):
                    h, kt = beats[j]
                    if h % 4 == 3:
                        r0, qt, kt_ = 32 * (h // 4), 0, 1
                        src = qk3
                    else:
                        r0, qt, kt_ = 32 * (h % 4), h // 4, 3 + h // 4
                        src = qkT
                    k0 = bn + kt * 128
                    sps = ps_a.tile([128, 2, NQC], F32, tag="mm", name="sps")
                    own = 0
                    for half in range(2):
                        qsl = slice(bn + half * NQC, bn + (half + 1) * NQC)
                        if dense[0]:
                            for g in range(2):
                                nc.tensor.matmul(
                                    sps[:, half, :],
                                    src[r0:r0 + 32, kt_, g, k0:k0 + 128],
                                    src[r0:r0 + 32, qt, g, qsl],
                                    start=(g == 0), stop=(g == 1))
                            own += EFF_SPAIR_DENSE // 2
                        else:
                            nc.tensor.matmul(
                                sps[:, half, :],
                                src[r0:r0 + 32, kt_, :, k0:k0 + 128],
                                src[r0:r0 + 32, qt, :, qsl],
                                perf_mode=mybir.MatmulPerfMode.DoubleRow,
                                start=True, stop=True)
                            own += EFF_SPAIR // 2
                    sps_of[j] = sps
                    return own

                def emit_pv(j):
                    h, kt = beats[j]
                    pt = pt_of.pop(j)
                    if kt == 0:
                        pvps_of[h] = [
                            ps_pv.tile([D + 1, NQC], F32, tag="pv",
                                       name="pvp") for _ in range(2)]
                    pvps = pvps_of[h]
                    for half in range(2):
                        nc.tensor.matmul(
                            pvps[half], v4[:, b * NK + kt, h, :],
                            pt[:, half, :],
                            start=(kt == 0), stop=(kt == NK - 1))
                    if kt == NK - 1:
                        # stash rowsums (partition-0 staging, then DMA to
                        # row h) + unnormalized O^T; frees pvp for h+2
                        ar0, at = 64 * (h % 2), h // 2
                        stg = stgp.tile([1, 2, NQC], F32, name="stg")
                        for half in range(2):
                            pvp = pvps[half]
                            qsl = slice(bn + half * NQC,
                                        bn + (half + 1) * NQC)
                            nc.vector.tensor_copy(
                                stg[0:1, half, :], pvp[D:D + 1, :])
                            nc.vector.tensor_copy(
                                aoT[ar0:ar0 + 64, at, qsl], pvp[0:D, :])
                        nc.sync.dma_start(out=rsall[h:h + 1, :, :], in_=stg)
                        del pvps_of[h]

                debt = 0
                own_next = emit_scores(0)
                for j, (h, kt) in enumerate(beats):
                    own = own_next
                    # force-drain fill up to any deadline: the qkT pairs that
                    # beat j+1's scores read must have been written (their
                    # drains emitted) before the scores matmul is emitted.
                    key = deadlines.get(j + 1) if deadlines else None
                    if key is not None:
                        while key not in fill_done:
                            cols = next(fill, None)
                            if cols is None:
                                break
                            debt -= cols
                    own_next = emit_scores(j + 1) if j + 1 < nb else 0
                    pt = ptp.tile([128, 2, NQC], F16)
                    nc.scalar.activation(
                        pt, sps_of.pop(j),
                        mybir.ActivationFunctionType.Exp,
                        bias=zb, scale=float(SCALE))
                    pt_of[j] = pt
                    pv_emits = pv_sched.get(j, [])
                    own += EFF_PVPAIR * len(pv_emits)
                    debt += BEAT_EFF - own
                    # drain fill every other beat in double portions: fewer
                    # stationary-family switches per beat (each group-head
                    # matmul pays a ~110-170ns LDWEIGHTS bubble)
                    if j % 2 == 1:
                        while debt > 0:
                            cols = next(fill, None)
                            if cols is None:
                                debt = 0
                                break
                            debt -= cols
                    for p in pv_emits:
                        emit_pv(p)
                for j in range(nb, nb + 2):
                    for p in pv_sched.get(j, []):
                        emit_pv(p)

            def emit_recip(rsall):
                # in-place: elementwise custom-DVE op, saves an SBUF tile
                nc.vector.reciprocal_approx_fast(rsall, rsall)
                rc = rcp.tile([H, 2, NQC], F16, name="rc")
                nc.vector.tensor_copy(rc, rsall)
                return rc

            # ---- schedule ----
            # P0 (dense PE, double-buffered): QK(b0) head-0-3 fts + V(b0)
            for i, ft in enumerate((0, 1, 6, 7)):
                pool, tag = (ps_a, "mm") if i % 2 == 0 else (ps_f, "fill")
                run_all(gen_qk(0, ft, pool, tag, scalar_copy=True))
            for kt in range(NK):
                pool, tag = (ps_a, "mm") if kt % 2 == 0 else (ps_f, "fill")
                run_all(gen_v(0, kt, pool, tag, scalar_copy=(kt % 2 == 1)))
            # attention(b0): fill = QK(b0) heads 4-11 (deadline beats 32/64),
            # then QK(b1) + V(b1) (needed before attention(b1) starts)
            rsall0 = rap.tile([H, 2, NQC], F32, name="rsall")
            fill0 = chain(
                [gen_qk(0, ft, ps_f, "fill") for ft in (2, 3, 8)] +
                [tracked(gen_qk(0, 9, ps_f, "fill"), "b0h47")] +
                [gen_qk(0, ft, ps_f, "fill") for ft in (4, 5, 10)] +
                [tracked(gen_qk(0, 11, ps_f, "fill"), "b0h8B")] +
                [gen_qk(1, ft, ps_f, "fill") for ft in (0, 1, 6, 7)] +
                [gen_v(1, kt, ps_f, "fill") for kt in range(NK)])
            emit_attention(0, rsall0, fill0,
                           deadlines={32: "b0h47", 64: "b0h8B"})
            # recip chain resolves on DVE while the PE drains leftover fill
            rc0 = emit_recip(rsall0)
            run_all(fill0)
            bcsb0 = emit_recip_done = make_bc(rc0)
            # attention(b1): fill = QK(b1) heads 4-11 + norm(b0) + OP(b0)
            rsall1 = rap.tile([H, 2, NQC], F32, name="rsall")
            fill1 = chain(
                [gen_qk(1, ft, ps_f, "fill") for ft in (2, 3, 8)] +
                [tracked(gen_qk(1, 9, ps_f, "fill"), "b1h47")] +
                [gen_qk(1, ft, ps_f, "fill") for ft in (4, 5, 10)] +
                [tracked(gen_qk(1, 11, ps_f, "fill"), "b1h8B")] +
                [gen_bc(0, 0, t, bcsb0) for t in range(NE)] +
                [gen_op(0, kt, ps_f, "fill") for kt in (0, 1, 2, 3)] +
                [gen_bc(0, 1, t, bcsb0) for t in range(NE)] +
                [gen_op(0, kt, ps_f, "fill") for kt in (4, 5)])
            emit_attention(1, rsall1, fill1,
                           deadlines={32: "b1h47", 64: "b1h8B"})
            rc1 = emit_recip(rsall1)
            # OP(b0) tail + fill leftovers kept back: PE chews them while
            # the recip -> bc chain for b1 resolves (was a ~10us PE gap)
            run_all(fill1)
            run_all(gen_op(0, 6, ps_a, "mm"))
            bcsb1 = make_bc(rc1)
            run_all(gen_op(0, 7, ps_f, "fill", scalar_copy=True))
            # tail: norm(b1) + OP(b1), double-buffered, drains DVE/ACT
            for half in range(2):
                for t in range(NE):
                    run_all(gen_bc(1, half, t, bcsb1))
                for kt in range(4 * half, 4 * half + 4):
                    pool, tag = ((ps_a, "mm") if kt % 2 == 0
                                 else (ps_f, "fill"))
                    run_all(gen_op(1, kt, pool, tag,
                                   scalar_copy=(kt % 2 == 1)))

    return nc


_COMPILED = None


def get_compiled():
    global _COMPILED
    if _COMPILED is None:
        nc = build_nc()
        nc.compile()
        _COMPILED = nc
    return _COMPILED


def _perm_qk(Wq):
    """[EM or 1, H*D] head-major -> fp8 DoubleRow tile layout [.., 6*128].

    Output column ft*128 + c (ft = 2*p + g) holds head 4p + c//32,
    dim (c%32) + 32*g.
    """
    src = Wq.reshape(-1, H, D)
    tiles = []
    for p in range(3):
        for g in range(2):
            cols = [src[:, 4 * p + j, 32 * g:32 * (g + 1)] for j in range(4)]
            tiles.append(np.concatenate(cols, axis=1))
    return np.concatenate(tiles, axis=1)


def make_in_maps(x, W_qk, b_qk, W_v, b_v, W_proj, b_proj):
    """Host prep: deinterleave+permute W_qk, transpose x, cast fp16."""
    W_qk = np.asarray(W_qk, dtype=np.float32)
    # reference: col index = h*(2*D) + dd*2 + qk  (qk fastest)
    Wq = W_qk.reshape(EM, H, D, 2)[..., 0].reshape(EM, H * D)
    Wk = W_qk.reshape(EM, H, D, 2)[..., 1].reshape(EM, H * D)
    b_qk = np.asarray(b_qk, dtype=np.float32)
    bq = b_qk.reshape(H, D, 2)[..., 0].reshape(1, H * D)
    bk = b_qk.reshape(H, D, 2)[..., 1].reshape(1, H * D)
    Wq, Wk, bq, bk = _perm_qk(Wq), _perm_qk(Wk), _perm_qk(bq), _perm_qk(bk)
    wqk = np.ascontiguousarray(
        np.concatenate([Wq, Wk], axis=1)).astype(np.float16)
    # per-feature bias as [128, 12] columns (partition-major per tile)
    bqkc = np.ascontiguousarray(
        np.concatenate([bq, bk], axis=1).reshape(2 * NE, 128).T
    ).astype(np.float32)
    wv = np.asarray(W_v, dtype=np.float32).astype(np.float16)
    wp = np.asarray(W_proj, dtype=np.float32).astype(np.float16)
    sel = np.zeros((H, NE, 128), dtype=np.float16)
    for t in range(NE):
        sel[2 * t, t, 0:64] = 1.0
        sel[2 * t + 1, t, 64:128] = 1.0
    sel = np.ascontiguousarray(sel.reshape(H, NE * 128))
    xs = np.asarray(x, dtype=np.float32).reshape(NCORES, T, EM)
    return [
        {"xt": np.ascontiguousarray(xs[i].T).astype(np.float16),
         "wqk": wqk, "bqkc": bqkc, "wv": wv, "wp": wp, "sel": sel}
        for i in range(NCORES)
    ]


def kernel(x, W_qk, b_qk, W_v, b_v, W_proj, b_proj):
    nc = get_compiled()
    in_maps = make_in_maps(x, W_qk, b_qk, W_v, b_v, W_proj, b_proj)
    res = run_bass_kernel_spmd(
        nc, in_maps, core_ids=list(range(NCORES))).results
    out = np.stack([np.asarray(res[i]["out"]) for i in range(NCORES)], axis=0)
    out = out.reshape(B, N, EM).astype(np.float32)
    # b_v passes through attention (softmax rows sum to 1); b_proj direct.
    b_eff = (np.asarray(b_v, dtype=np.float64) @
             np.asarray(W_proj, dtype=np.float64) +
             np.asarray(b_proj, dtype=np.float64)).astype(np.float32)
    return out + b_eff


# revision 37
# speedup vs baseline: 1.0171x; 1.0171x over previous
"""Multi-head attention (B=16, N=1024, EM=768, H=12, d=64) on 8 TRN2 NeuronCores.

Strategy: data-parallel over batch (2 batches per core, zero collectives).
v2 schedule (vs v1 baseline at 533us): the whole kernel is organized so the
PE never idles (keeps the tensor engine in its high p-state) and the ACT
engine's exp chain is never gated by PE queue position:

  1. Scores-ahead software pipelining: the score matmuls for beat j+1 are
     emitted BEFORE exp(j), so when exp(j) ends, exp(j+1)'s input is long
     since ready -- exp never waits on beat-j fill/PV sitting ahead of it
     in the in-order PE queue (that wait was ~430ns/beat = ~40% of the
     baseline's attention time).
  2. Minimal pre-attention phase: only QK(b0) head-0-3 feature tiles +
     V(b0) run dense; attention(b0) starts ~20us in. All remaining
     projection work (QK(b0) heads 4-11, QK(b1), V(b1), then OP(b0)+
     norm(b0) during attention(b1)) is deadline-ordered fill inside the
     exp-gated beats.
  3. Per-beat fill debt is budgeted in ns-equivalent units including
     LDWEIGHTS cost so beats stay just above the exp duration -- PE
     stays continuously busy (full 2.4GHz p-state) without overpacking.

Numerics identical to baseline (fp8 DoubleRow scores from host-permuted
W_qk, fp16 everything else, exp without max-subtraction, rowsums via a
constant-1 V column, b_v/b_proj folded on host). fp8 anywhere else
(V proj, PV, OP) measurably blows the 2e-2 budget: attention output is a
near-uniform average of random-sign values, so per-element quantization
noise passes through at full magnitude instead of averaging out.
"""

import sys

if "/opt/trn_rl_repo" not in sys.path:
    sys.path.insert(0, "/opt/trn_rl_repo")

import numpy as np

from concourse import bacc, mybir, tile
from concourse.bass_utils import run_bass_kernel_spmd

F8 = mybir.dt.float8e4
F16 = mybir.dt.float16
F32 = mybir.dt.float32

B, N, EM = 16, 1024, 768
H, D = 12, 64
NCORES = 8
BL = B // NCORES          # batches per core
T = BL * N                # tokens per core
NT = T // 128             # 16 token tiles
NE = EM // 128            # 6 em tiles
NQC = 512                 # q-chunk width
NK = N // 128             # 8 k-tiles per batch
SCALE = 1.0 / np.sqrt(np.float32(D))

# PE work estimates in "effective columns" (~ns * 2.4), calibrated from the
# v1 hardware trace (per-matmul ~46ns fixed tax + stationary-switch bubbles).
EFF_MM512 = 625           # 512-col fp16 matmul (~260ns)
EFF_MM256 = 350
EFF_SPAIR = 1650          # score pair, both halves (~690ns)
EFF_SPAIR_DENSE = 2250    # dense fp8 score pair (2 accumulating mm per half)
EFF_PVPAIR = 1245         # fp16 PV pair (~520ns)
EFF_BC = 1000             # norm broadcast matmul (~420ns)
BEAT_EFF = 4400           # per-beat PE budget (~1830ns)


def build_nc():
    nc = bacc.Bacc("TRN2", target_bir_lowering=False, debug=False,
                   num_devices=NCORES)
    xt_d = nc.dram_tensor("xt", [EM, T], F16, kind="ExternalInput").ap()
    wqk_d = nc.dram_tensor("wqk", [EM, 2 * EM], F16, kind="ExternalInput").ap()
    bqkc_d = nc.dram_tensor("bqkc", [128, 2 * NE], F32,
                            kind="ExternalInput").ap()
    wv_d = nc.dram_tensor("wv", [EM, EM], F16, kind="ExternalInput").ap()
    wp_d = nc.dram_tensor("wp", [EM, EM], F16, kind="ExternalInput").ap()
    sel_d = nc.dram_tensor("sel", [H, NE * 128], F16, kind="ExternalInput").ap()
    out_d = nc.dram_tensor("out", [T, EM], F16, kind="ExternalOutput").ap()

    with tile.TileContext(nc) as tc:
        with (
            tc.tile_pool(name="big", bufs=1) as big,
            tc.tile_pool(name="ptp", bufs=5) as ptp,
            tc.tile_pool(name="rap", bufs=1) as rap,
            tc.tile_pool(name="stgp", bufs=2) as stgp,
            tc.tile_pool(name="rcp", bufs=1) as rcp,
            tc.tile_pool(name="osb", bufs=2) as osbp,
            tc.tile_pool(name="bcsp", bufs=1) as bcsp,
            tc.tile_pool(name="ps_a", bufs=2, space="PSUM") as ps_a,
            tc.tile_pool(name="ps_pv", bufs=2, space="PSUM") as ps_pv,
            tc.tile_pool(name="ps_f", bufs=1, space="PSUM") as ps_f,
        ):
            # ---- constants ----
            sel = big.tile([H, NE, 128], F16)
            nc.sync.dma_start(out=sel, in_=sel_d)
            zb = big.tile([128, 1], F32)
            nc.vector.memset(zb, 0.0)

            wqk_sb = big.tile([128, NE, 2 * EM], F16)
            wv_sb = big.tile([128, NE, EM], F16)
            wp_sb = big.tile([128, NE, EM], F16)
            bqkc = big.tile([128, 2 * NE], F32)
            xT = big.tile([128, NE, T], F16)
            nc.sync.dma_start(out=bqkc, in_=bqkc_d)
            # DMA priority: P0's needs first (QK(b0) fts 0,1,6,7 + V(b0)),
            # then the rest in fill-deadline order. Issue from three
            # sequencers (sync/scalar/vector) so the queues stream in
            # parallel and the first QK matmul can start sooner.
            for e in range(NE):
                sl = slice(e * 128, (e + 1) * 128)
                nc.sync.dma_start(out=wqk_sb[:, e, 0:256],
                                  in_=wqk_d[sl, 0:256])
                nc.sync.dma_start(out=wqk_sb[:, e, 768:1024],
                                  in_=wqk_d[sl, 768:1024])
                nc.scalar.dma_start(out=xT[:, e, 0:N], in_=xt_d[sl, 0:N])
            for e in range(NE):
                sl = slice(e * 128, (e + 1) * 128)
                nc.sync.dma_start(out=wv_sb[:, e, :], in_=wv_d[sl, :])
                nc.scalar.dma_start(out=xT[:, e, N:T], in_=xt_d[sl, N:T])
            for e in range(NE):
                sl = slice(e * 128, (e + 1) * 128)
                nc.sync.dma_start(out=wqk_sb[:, e, 256:768],
                                  in_=wqk_d[sl, 256:768])
                nc.sync.dma_start(out=wqk_sb[:, e, 1024:1536],
                                  in_=wqk_d[sl, 1024:1536])
                nc.sync.dma_start(out=wp_sb[:, e, :], in_=wp_d[sl, :])

            # [32-row group, qk-pair, dgroup, tok]; head h lives at rows
            # 32*(h%4) of pair h//4 (Q) / 3+h//4 (K), d split as 2x32.
            # Matmul operand base partitions must be 0/32/64, so the
            # rows-96 head is DMA-duplicated into qk3 (base 0).
            qkT = big.tile([128, 2 * NE, 2, T], F8)
            qk3 = big.tile([128, 2, 2, T], F8)
            v4 = big.tile([128, NT, H, D + 1], F16)
            nc.vector.memset(v4[:, :, :, D:D + 1], 1.0)
            aoT = big.tile([128, NE, T], F16)

            def gen_qk(b, ft, pool, tag, scalar_copy=False):
                """QK projection for one feature tile; yields eff-cols."""
                bn = b * N
                ps = pool.tile([128, 2, NQC], F32, tag=tag, name="fqk")
                # e-outer: both halves share each e's stationary (one LDW)
                for e in range(NE):
                    for half in range(2):
                        csl = slice(bn + half * NQC, bn + (half + 1) * NQC)
                        nc.tensor.matmul(
                            ps[:, half, :],
                            wqk_sb[:, e, ft * 128:(ft + 1) * 128],
                            xT[:, e, csl],
                            start=(e == 0), stop=(e == NE - 1))
                        yield EFF_MM512
                for half in range(2):
                    csl = slice(bn + half * NQC, bn + (half + 1) * NQC)
                    dst = qkT[:, ft // 2, ft % 2, csl]
                    if scalar_copy and half == 1:
                        nc.scalar.activation(
                            dst, ps[:, half, :],
                            mybir.ActivationFunctionType.Identity,
                            bias=bqkc[:, ft:ft + 1])
                    else:
                        nc.vector.tensor_scalar_add(
                            dst, ps[:, half, :], bqkc[:, ft:ft + 1])
                if ft % 2 == 1:
                    fp = ft // 2
                    g0 = 32 * (fp % 3)
                    bsl = slice(bn, bn + N)
                    nc.sync.dma_start(
                        out=qk3[g0:g0 + 32, fp // 3, :, bsl],
                        in_=qkT[96:128, fp, :, bsl])

            def gen_v(b, kt, pool, tag, scalar_copy=False):
                tt = b * NK + kt
                tsl = slice(tt * 128, (tt + 1) * 128)
                ps = pool.tile([128, H, D], F32, tag=tag, name="fv")
                for e in range(NE):
                    for h0, h1 in ((0, 8), (8, 12)):
                        fsl = slice(h0 * D, h1 * D)
                        nc.tensor.matmul(
                            ps[:, h0:h1, :], xT[:, e, tsl],
                            wv_sb[:, e, fsl],
                            start=(e == 0), stop=(e == NE - 1))
                        yield EFF_MM512 if h1 - h0 == 8 else EFF_MM256
                if scalar_copy:
                    nc.scalar.copy(v4[:, tt, :, 0:D], ps)
                else:
                    nc.vector.tensor_copy(v4[:, tt, :, 0:D], ps)

            def gen_op(b, kt, pool, tag, scalar_copy=False):
                tt = b * NK + kt
                tsl = slice(tt * 128, (tt + 1) * 128)
                ps = pool.tile([128, EM], F32, tag=tag, name="fop")
                for dv in range(NE):
                    for c0, c1 in ((0, 512), (512, 768)):
                        nc.tensor.matmul(
                            ps[:, c0:c1], aoT[:, dv, tsl],
                            wp_sb[:, dv, c0:c1],
                            start=(dv == 0), stop=(dv == NE - 1))
                        yield EFF_MM512 if c1 - c0 == 512 else EFF_MM256
                osb = osbp.tile([128, EM], F16)
                if scalar_copy:
                    nc.scalar.copy(osb, ps)
                else:
                    nc.vector.tensor_copy(osb, ps)
                nc.sync.dma_start(out=out_d[tsl, :], in_=osb)

            def make_bc(rc):
                """All 12 recip-broadcast tiles via PE sel-matmuls, drained
                to fp16 SBUF in the idle phase-boundary window. The fill-time
                norm granules are then a pure DVE multiply (no PSUM held
                across the fill pipeline)."""
                bcsb = bcsp.tile([128, 2, NE, NQC], F16, name="bcsb")
                for i, (half, t) in enumerate(
                        [(hh, tt) for hh in range(2) for tt in range(NE)]):
                    pool, tag = ((ps_a, "mm") if i % 2 == 0
                                 else (ps_f, "fill"))
                    bc = pool.tile([128, NQC], F32, tag=tag, name="fbc")
                    nc.tensor.matmul(bc, sel[:, t, :], rc[:, half, :],
                                     start=True, stop=True)
                    if i % 2 == 1:
                        nc.scalar.copy(bcsb[:, half, t, :], bc)
                    else:
                        nc.vector.tensor_copy(bcsb[:, half, t, :], bc)
                return bcsb

            def gen_make_bc(rc, bcsb):
                for i, (half, t) in enumerate(
                        [(hh, tt) for hh in range(2) for tt in range(NE)]):
                    pool, tag = ((ps_a, "mm") if i % 2 == 0
                                 else (ps_f, "fill"))
                    bc = pool.tile([128, NQC], F32, tag=tag, name="fbc")
                    nc.tensor.matmul(bc, sel[:, t, :], rc[:, half, :],
                                     start=True, stop=True)
                    yield 650
                    if i % 2 == 1:
                        nc.scalar.copy(bcsb[:, half, t, :], bc)
                    else:
                        nc.vector.tensor_copy(bcsb[:, half, t, :], bc)

            def gen_bc(b, half, t, bcsb):
                bn = b * N
                qsl = slice(bn + half * NQC, bn + (half + 1) * NQC)
                yield 50
                dst = aoT[:, t, qsl]
                nc.vector.tensor_mul(dst, dst, bcsb[:, half, t, :])

            fill_done = set()

            def chain(gens):
                for g in gens:
                    yield from g

            def tracked(gen, key):
                """Marks `key` done once the generator (incl. its trailing
                drain emissions) has fully run."""
                yield from gen
                fill_done.add(key)

            def run_all(g):
                for _ in g:
                    pass

            def emit_attention(b, rsall, fill, deadlines=()):
                """Software-pipelined beats: scores for beat j+1 are emitted
                before exp(j) so the ACT exp chain is never queued behind
                beat-j fill/PV work on the in-order PE."""
                bn = b * N
                beats = [(h, kt) for h in range(H) for kt in range(NK)]
                nb = len(beats)
                sps_of = {}
                pt_of = {}
                pvps_of = {}
                dense = [False]   # flips when fill runs dry: keep PE padded
                # PV(h,kt) is emitted at loop 8h+PV_AT[kt]: lag >=2 from its
                # exp (decouples ACT->PE), and head h's first pvp write lands
                # 3 beats after head h-1's drain was queued -- the 2-buf pvp
                # pool never stalls the PE at head boundaries.
                PV_AT = {0: 4, 1: 4, 2: 5, 3: 5, 4: 6, 5: 7, 6: 8, 7: 9}
                pv_sched = {}
                for hh in range(H):
                    for kk in range(NK):
                        pv_sched.setdefault(
                            8 * hh + PV_AT[kk], []).append(8 * hh + kk)

                def emit_scores(j):
                    h, kt = beats[j]
                    if h % 4 == 3:
                        r0, qt, kt_ = 32 * (h // 4), 0, 1
                        src = qk3
                    else:
                        r0, qt, kt_ = 32 * (h % 4), h // 4, 3 + h // 4
                        src = qkT
                    k0 = bn + kt * 128
                    sps = ps_a.tile([128, 2, NQC], F32, tag="mm", name="sps")
                    own = 0
                    for half in range(2):
                        qsl = slice(bn + half * NQC, bn + (half + 1) * NQC)
                        if dense[0]:
                            for g in range(2):
                                nc.tensor.matmul(
                                    sps[:, half, :],
                                    src[r0:r0 + 32, kt_, g, k0:k0 + 128],
                                    src[r0:r0 + 32, qt, g, qsl],
                                    start=(g == 0), stop=(g == 1))
                            own += EFF_SPAIR_DENSE // 2
                        else:
                            nc.tensor.matmul(
                                sps[:, half, :],
                                src[r0:r0 + 32, kt_, :, k0:k0 + 128],
                                src[r0:r0 + 32, qt, :, qsl],
                                perf_mode=mybir.MatmulPerfMode.DoubleRow,
                                start=True, stop=True)
                            own += EFF_SPAIR // 2
                    sps_of[j] = sps
                    return own

                def emit_pv(j):
                    h, kt = beats[j]
                    pt = pt_of.pop(j)
                    if kt == 0:
                        pvps_of[h] = [
                            ps_pv.tile([D + 1, NQC], F32, tag="pv",
                                       name="pvp") for _ in range(2)]
                    pvps = pvps_of[h]
                    for half in range(2):
                        nc.tensor.matmul(
                            pvps[half], v4[:, b * NK + kt, h, :],
                            pt[:, half, :],
                            start=(kt == 0), stop=(kt == NK - 1))
                    if kt == NK - 1:
                        # stash rowsums (partition-0 staging, then DMA to
                        # row h) + unnormalized O^T; frees pvp for h+2
                        ar0, at = 64 * (h % 2), h // 2
                        stg = stgp.tile([1, 2, NQC], F32, name="stg")
                        for half in range(2):
                            pvp = pvps[half]
                            qsl = slice(bn + half * NQC,
                                        bn + (half + 1) * NQC)
                            nc.vector.tensor_copy(
                                stg[0:1, half, :], pvp[D:D + 1, :])
                            nc.vector.tensor_copy(
                                aoT[ar0:ar0 + 64, at, qsl], pvp[0:D, :])
                        nc.sync.dma_start(out=rsall[h:h + 1, :, :], in_=stg)
                        del pvps_of[h]

                debt = 0
                own_next = emit_scores(0)
                for j, (h, kt) in enumerate(beats):
                    own = own_next
                    # force-drain fill up to any deadline: the qkT pairs that
                    # beat j+1's scores read must have been written (their
                    # drains emitted) before the scores matmul is emitted.
                    key = deadlines.get(j + 1) if deadlines else None
                    if key is not None:
                        while key not in fill_done:
                            cols = next(fill, None)
                            if cols is None:
                                break
                            debt -= cols
                    own_next = emit_scores(j + 1) if j + 1 < nb else 0
                    pt = ptp.tile([128, 2, NQC], F16)
                    nc.scalar.activation(
                        pt, sps_of.pop(j),
                        mybir.ActivationFunctionType.Exp,
                        bias=zb, scale=float(SCALE))
                    pt_of[j] = pt
                    pv_emits = pv_sched.get(j, [])
                    own += EFF_PVPAIR * len(pv_emits)
                    debt += BEAT_EFF - own
                    # drain fill every other beat in double portions: fewer
                    # stationary-family switches per beat (each group-head
                    # matmul pays a ~110-170ns LDWEIGHTS bubble)
                    if j % 2 == 1:
                        while debt > 0:
                            cols = next(fill, None)
                            if cols is None:
                                debt = 0
                                break
                            debt -= cols
                    for p in pv_emits:
                        emit_pv(p)
                for j in range(nb, nb + 2):
                    for p in pv_sched.get(j, []):
                        emit_pv(p)

            def emit_recip(rsall):
                # in-place: elementwise custom-DVE op, saves an SBUF tile
                nc.vector.reciprocal_approx_fast(rsall, rsall)
                rc = rcp.tile([H, 2, NQC], F16, name="rc")
                nc.vector.tensor_copy(rc, rsall)
                return rc

            # ---- schedule ----
            # P0 (dense PE, double-buffered): QK(b0) head-0-3 fts + V(b0)
            for i, ft in enumerate((0, 1, 6, 7)):
                pool, tag = (ps_a, "mm") if i % 2 == 0 else (ps_f, "fill")
                run_all(gen_qk(0, ft, pool, tag, scalar_copy=True))
            for kt in range(NK):
                pool, tag = (ps_a, "mm") if kt % 2 == 0 else (ps_f, "fill")
                run_all(gen_v(0, kt, pool, tag, scalar_copy=(kt % 2 == 1)))
            # attention(b0): fill = QK(b0) heads 4-11 (deadline beats 32/64),
            # then QK(b1) + V(b1) (needed before attention(b1) starts)
            rsall0 = rap.tile([H, 2, NQC], F32, name="rsall")
            fill0 = chain(
                [gen_qk(0, ft, ps_f, "fill") for ft in (2, 3, 8)] +
                [tracked(gen_qk(0, 9, ps_f, "fill"), "b0h47")] +
                [gen_qk(0, ft, ps_f, "fill") for ft in (4, 5, 10)] +
                [tracked(gen_qk(0, 11, ps_f, "fill"), "b0h8B")] +
                [gen_qk(1, ft, ps_f, "fill") for ft in (0, 1, 6, 7)] +
                [gen_v(1, kt, ps_f, "fill") for kt in range(NK)])
            emit_attention(0, rsall0, fill0,
                           deadlines={32: "b0h47", 64: "b0h8B"})
            # recip chain resolves on DVE while the PE drains leftover fill
            rc0 = emit_recip(rsall0)
            run_all(fill0)
            bcsb0 = bcsp.tile([128, 2, NE, NQC], F16, name="bcsb")
            # attention(b1): fill = QK(b1) heads 4-11 + norm(b0) + OP(b0)
            rsall1 = rap.tile([H, 2, NQC], F32, name="rsall")
            fill1 = chain(
                [gen_make_bc(rc0, bcsb0)] +
                [gen_qk(1, ft, ps_f, "fill") for ft in (2, 3, 8)] +
                [tracked(gen_qk(1, 9, ps_f, "fill"), "b1h47")] +
                [gen_qk(1, ft, ps_f, "fill") for ft in (4, 5, 10)] +
                [tracked(gen_qk(1, 11, ps_f, "fill"), "b1h8B")] +
                [gen_bc(0, 0, t, bcsb0) for t in range(NE)] +
                [gen_op(0, kt, ps_f, "fill") for kt in (0, 1, 2, 3)] +
                [gen_bc(0, 1, t, bcsb0) for t in range(NE)] +
                [gen_op(0, kt, ps_f, "fill") for kt in (4, 5)])
            emit_attention(1, rsall1, fill1,
                           deadlines={32: "b1h47", 64: "b1h8B"})
            rc1 = emit_recip(rsall1)
            # OP(b0) tail + fill leftovers kept back: PE chews them while
            # the recip -> bc chain for b1 resolves (was a ~10us PE gap)
            run_all(fill1)
            run_all(gen_op(0, 6, ps_a, "mm"))
            bcsb1 = make_bc(rc1)
            run_all(gen_op(0, 7, ps_f, "fill", scalar_copy=True))
            # tail: norm(b1) + OP(b1), double-buffered, drains DVE/ACT
            for half in range(2):
                for t in range(NE):
                    run_all(gen_bc(1, half, t, bcsb1))
                for kt in range(4 * half, 4 * half + 4):
                    pool, tag = ((ps_a, "mm") if kt % 2 == 0
                                 else (ps_f, "fill"))
                    run_all(gen_op(1, kt, pool, tag,
                                   scalar_copy=(kt % 2 == 1)))

    return nc


_COMPILED = None


def get_compiled():
    global _COMPILED
    if _COMPILED is None:
        nc = build_nc()
        nc.compile()
        _COMPILED = nc
    return _COMPILED


def _perm_qk(Wq):
    """[EM or 1, H*D] head-major -> fp8 DoubleRow tile layout [.., 6*128].

    Output column ft*128 + c (ft = 2*p + g) holds head 4p + c//32,
    dim (c%32) + 32*g.
    """
    src = Wq.reshape(-1, H, D)
    tiles = []
    for p in range(3):
        for g in range(2):
            cols = [src[:, 4 * p + j, 32 * g:32 * (g + 1)] for j in range(4)]
            tiles.append(np.concatenate(cols, axis=1))
    return np.concatenate(tiles, axis=1)


def make_in_maps(x, W_qk, b_qk, W_v, b_v, W_proj, b_proj):
    """Host prep: deinterleave+permute W_qk, transpose x, cast fp16."""
    W_qk = np.asarray(W_qk, dtype=np.float32)
    # reference: col index = h*(2*D) + dd*2 + qk  (qk fastest)
    Wq = W_qk.reshape(EM, H, D, 2)[..., 0].reshape(EM, H * D)
    Wk = W_qk.reshape(EM, H, D, 2)[..., 1].reshape(EM, H * D)
    b_qk = np.asarray(b_qk, dtype=np.float32)
    bq = b_qk.reshape(H, D, 2)[..., 0].reshape(1, H * D)
    bk = b_qk.reshape(H, D, 2)[..., 1].reshape(1, H * D)
    Wq, Wk, bq, bk = _perm_qk(Wq), _perm_qk(Wk), _perm_qk(bq), _perm_qk(bk)
    wqk = np.ascontiguousarray(
        np.concatenate([Wq, Wk], axis=1)).astype(np.float16)
    # per-feature bias as [128, 12] columns (partition-major per tile)
    bqkc = np.ascontiguousarray(
        np.concatenate([bq, bk], axis=1).reshape(2 * NE, 128).T
    ).astype(np.float32)
    wv = np.asarray(W_v, dtype=np.float32).astype(np.float16)
    wp = np.asarray(W_proj, dtype=np.float32).astype(np.float16)
    sel = np.zeros((H, NE, 128), dtype=np.float16)
    for t in range(NE):
        sel[2 * t, t, 0:64] = 1.0
        sel[2 * t + 1, t, 64:128] = 1.0
    sel = np.ascontiguousarray(sel.reshape(H, NE * 128))
    xs = np.asarray(x, dtype=np.float32).reshape(NCORES, T, EM)
    return [
        {"xt": np.ascontiguousarray(xs[i].T).astype(np.float16),
         "wqk": wqk, "bqkc": bqkc, "wv": wv, "wp": wp, "sel": sel}
        for i in range(NCORES)
    ]


def kernel(x, W_qk, b_qk, W_v, b_v, W_proj, b_proj):
    nc = get_compiled()
    in_maps = make_in_maps(x, W_qk, b_qk, W_v, b_v, W_proj, b_proj)
    res = run_bass_kernel_spmd(
        nc, in_maps, core_ids=list(range(NCORES))).results
    out = np.stack([np.asarray(res[i]["out"]) for i in range(NCORES)], axis=0)
    out = out.reshape(B, N, EM).astype(np.float32)
    # b_v passes through attention (softmax rows sum to 1); b_proj direct.
    b_eff = (np.asarray(b_v, dtype=np.float64) @
             np.asarray(W_proj, dtype=np.float64) +
             np.asarray(b_proj, dtype=np.float64)).astype(np.float32)
    return out + b_eff
